# revision 3
# baseline (speedup 1.0000x reference)
"""GQA attention kernel for 8 trn2 NeuronCores (Bass/Tile, SPMD) — v9.

Problem: X[2,2048,2048] fp32, Wq[2048,2048], Wk/Wv[2048,512], Wo[2048,2048].
  q/k/v proj -> GQA attention (32 Q heads, 8 KV heads, head_dim 64, no mask)
  -> out proj.

Sharding (8 cores): core c handles batch b=c//4 and query heads
[8j, 8j+8) with KV heads {2j, 2j+1} where j=c%4.  Heads are processed in
pairs (p, p+4): head p (KV head 2j) on partitions 0-63, head p+4
(KV head 2j+1) on partitions 64-127.

Design (everything measured on HW via microbenchmarks / NTFF traces):
  - all weights + X^T converted to bf16 AND pre-transposed to
    partition-major layout on the host, so every DMA is contiguous >=4KB
    per partition row (the naive rearranged loads generated 256B
    descriptors and stalled phase A by ~20us).
  - every matmul is bf16/bf16: bf16 stationaries let LDWEIGHTS overlap
    the matmul stream (213ns per 512-wide MM vs ~372ns self-loading f32r).
  - scores use tile_position row-packing: the two heads' K=64 matmuls run
    concurrently in the two PE row halves (~231ns per packed pair).
  - softmax exp on ACT is the attention bottleneck (256 x [128,1024]
    exps ~= 261us busy); the whole schedule is built to keep ACT fed:
      * per 2-kt group: 4 score MMs -> 2 exps -> (4 PV MMs deferred by 2
        groups) so the in-order PE queue never holds an ACT-blocked PV
        ahead of ready score matmuls, and the PE's half-array<->full-array
        stationary reconfiguration happens once per group (~200ns).
      * the PV pipeline runs continuously across a pair's 4 query blocks.
      * Q-projection of pair 0 is folded into phase A (K/V projection);
        Q-projection of pair p+1 is woven 2-4 matmuls per group into pair
        p's attention stream, borrowing the o-ring PSUM buffers, so ACT
        never sees a projection lump.
  - softmax normalize: rowsum via fused ones-column in the PV stationary;
    reciprocal on DVE; partition-broadcast on the (otherwise idle) Pool
    engine; o-scaling mul on DVE -> no PE work in the tail.
  - per-(pair, seq-half) chunked AllGathers (8 total) overlap attention;
    oproj orders pair-3 contraction chunks last so the final gather hides.

PSUM: phase A: kv[128,1024]x2 + q[128,1024]x2;  phase B: s[128,1024]
bufs=2 + o-tag [128,1024] bufs=2 (o accumulators and interleaved Q-proj
tiles share this ring);  phase C: psy[128,512] x4 tags x bufs=2.
"""

import os
import sys
import types
from contextlib import ExitStack

import numpy as np

_HIDDEN = 2048
_SEQ = 2048
_BATCH = 2
_NH = 32  # query heads
_NKV = 8
_HD = 64  # head dim
_NCORES = 8

_KC = _HIDDEN // 128  # 16 contraction chunks for proj/oproj
_NT = _SEQ // 512  # 4 query tiles of 512
_ST = _SEQ // 128  # 16 key chunks of 128
_NPAIR = 4  # head pairs per core

_ORDER = [0, 4, 1, 5, 2, 6, 3, 7]  # within-core local head order (pairing)


def _install_ntff_hook():
    """antenv in this image lacks axon_hooks; synthesize it so the axon
    NTFF profiling path works when tracing is requested."""
    try:
        import antenv

        try:
            from antenv import axon_hooks  # noqa: F401

            return
        except ImportError:
            pass
        mod = types.ModuleType("antenv.axon_hooks")
        mod._hook = None
        mod.set_axon_ntff_profile_hook = lambda h: setattr(mod, "_hook", h)
        mod.get_axon_ntff_profile_hook = lambda: mod._hook
        sys.modules["antenv.axon_hooks"] = mod
        antenv.axon_hooks = mod
        from trn_agent_boot.trn_boot import _ntff_profile_via_ctypes

        so = "/opt/axon/libaxon_pjrt.so"
        if os.path.exists(so):
            mod.set_axon_ntff_profile_hook(_ntff_profile_via_ctypes(so))
    except Exception:
        pass


_install_ntff_hook()

import concourse.bass as bass  # noqa: E402
import concourse.tile as tile  # noqa: E402
from concourse import bacc, bass_utils, mybir  # noqa: E402
from concourse.bass_utils import run_bass_kernel_spmd  # noqa: E402
from concourse.masks import make_identity  # noqa: E402

# no S3 bucket in this container; keep trace artifacts local
bass_utils.upload_artifacts = lambda tmpdir: tmpdir

F32 = mybir.dt.float32
BF16 = mybir.dt.bfloat16
U16 = mybir.dt.uint16

_nc_cache = None
_last_results = None


def _build():
    nc = bacc.Bacc("TRN2", target_bir_lowering=False, debug=False, num_devices=8)

    # all inputs host-prepared in partition-major [128, kc*cols] bf16 bits
    xt_d = nc.declare_dram_parameter("xt", [128, _KC * _SEQ], U16, isOutput=False)
    wq_d = nc.declare_dram_parameter("wq", [128, _KC * 512], U16, isOutput=False)
    wk_d = nc.declare_dram_parameter("wk", [128, _KC * 128], U16, isOutput=False)
    wv_d = nc.declare_dram_parameter("wv", [128, _KC * 128], U16, isOutput=False)
    wo_d = nc.declare_dram_parameter("wo", [128, _KC * 512], U16, isOutput=False)
    yt_d = nc.declare_dram_parameter("yt", [512, _SEQ], F32, isOutput=True)

    # per-(pair, seq-half) A^T chunks: local [128,1024] -> gathered [512,1024]
    at_loc = [
        [nc.dram_tensor(f"at_loc{p}_{h}", [128, 1024], BF16) for h in range(2)]
        for p in range(_NPAIR)
    ]
    at_full = [
        [nc.dram_tensor(f"at_full{p}_{h}", [512, 1024], BF16) for h in range(2)]
        for p in range(_NPAIR)
    ]

    with (
        tile.TileContext(nc) as tc,
        ExitStack() as ctx,
        nc.allow_low_precision(reason="bf16 matmuls throughout; tol 2e-2"),
    ):
        const = ctx.enter_context(tc.tile_pool(name="const", bufs=1))

        # ---- persistent SBUF tensors -------------------------------------
        wq_sb = const.tile([128, _NPAIR, _KC, 128], BF16, tag="wq")
        wk_sb = const.tile([128, _KC, 128], BF16, tag="wk")
        wv_sb = const.tile([128, _KC, 128], BF16, tag="wv")
        wo_sb = const.tile([128, _KC, 512], BF16, tag="wo")
        xt_sb = const.tile([128, _KC, _SEQ], BF16, tag="xt")

        # contiguous partition-major loads; wk/wv/wq before xt (needed first)
        nc.scalar.dma_start(out=wk_sb, in_=wk_d[:, :].bitcast(BF16))
        nc.scalar.dma_start(out=wv_sb, in_=wv_d[:, :].bitcast(BF16))
        nc.scalar.dma_start(
            out=wq_sb[:, 0, :, :], in_=wq_d[:, 0:2048].bitcast(BF16)
        )
        for kc in range(_KC):
            nc.sync.dma_start(
                out=xt_sb[:, kc, :],
                in_=xt_d[:, kc * _SEQ : (kc + 1) * _SEQ].bitcast(BF16),
            )
        for p in range(1, _NPAIR):
            nc.scalar.dma_start(
                out=wq_sb[:, p, :, :],
                in_=wq_d[:, p * 2048 : (p + 1) * 2048].bitcast(BF16),
            )
        nc.scalar.dma_start(out=wo_sb, in_=wo_d[:, :].bitcast(BF16))

        ident = const.tile([128, 128], BF16, tag="ident")
        make_identity(nc, ident)

        qt_sb = [
            const.tile([128, _SEQ], BF16, tag=f"qt{p}", name=f"qt{p}")
            for p in range(_NPAIR)
        ]
        kt_sb = const.tile([128, _SEQ], BF16, tag="kt")
        vt_sb = const.tile([128, _SEQ], BF16, tag="vt")
        # [V_A | 1 | V_B | 1] per key chunk
        vone = const.tile([128, _ST, 130], BF16, tag="vone")
        nc.vector.memset(vone[:, :, 64], 1.0)
        nc.vector.memset(vone[:, :, 129], 1.0)

        qsl_A = slice(0, 64)
        qsl_B = slice(64, 128)

        # ---- phase A: K/V projections + pair-0 Q proj + V transposes ----
        # kc-major seq-halves: matmuls start as soon as the first xt chunk
        # lands; pair-0's Q projection rides along (same xt chunks, one
        # extra stationary), so attention can start right after phase A.
        with tc.tile_pool(name="ps_a", bufs=1, space="PSUM") as ps_a:
            for hh in range(2):
                kvs = [
                    ps_a.tile([128, 1024], F32, tag="kv", bufs=2, name=f"kv{hh}{i}")
                    for i in range(2)
                ]
                psq0 = ps_a.tile([128, 1024], F32, tag="q", bufs=2, name=f"q{hh}")
                for kc in range(_KC):
                    st, sp = kc == 0, kc == _KC - 1
                    for i in range(2):
                        n = 2 * hh + i
                        nsl = slice(n * 512, (n + 1) * 512)
                        nc.tensor.matmul(
                            kvs[i][:, 0:512], wk_sb[:, kc, :], xt_sb[:, kc, nsl],
                            start=st, stop=sp,
                        )
                        nc.tensor.matmul(
                            kvs[i][:, 512:1024], wv_sb[:, kc, :], xt_sb[:, kc, nsl],
                            start=st, stop=sp,
                        )
                        nc.tensor.matmul(
                            psq0[:, i * 512 : (i + 1) * 512],
                            wq_sb[:, 0, kc, :],
                            xt_sb[:, kc, nsl],
                            start=st,
                            stop=sp,
                        )
                for i in range(2):
                    n = 2 * hh + i
                    nsl = slice(n * 512, (n + 1) * 512)
                    nc.vector.tensor_copy(kt_sb[:, nsl], kvs[i][:, 0:512])
                    nc.vector.tensor_copy(vt_sb[:, nsl], kvs[i][:, 512:1024])
                nc.vector.tensor_copy(
                    qt_sb[0][:, hh * 1024 : (hh + 1) * 1024], psq0
                )
        with tc.tile_pool(name="ps_tp", bufs=4, space="PSUM") as ps_tp:
            for sc in range(_ST):
                tp = ps_tp.tile([128, 128], BF16, tag="tp")
                nc.tensor.transpose(tp, vt_sb[:, sc * 128 : (sc + 1) * 128], ident)
                nc.vector.tensor_copy(vone[:, sc, 0:64], tp[:, 0:64])
                nc.vector.tensor_copy(vone[:, sc, 65:129], tp[:, 64:128])

        # ---- phase B: attention, with next pair's Q proj woven in -------
        with (
            tc.tile_pool(name="pp_pool", bufs=8) as pp_pool,
            tc.tile_pool(name="nrm_pool", bufs=2) as nrm_pool,
            tc.tile_pool(name="ps_b", bufs=1, space="PSUM") as ps_b,
        ):
            for p in range(_NPAIR):

                def pv(o_pair, kt, pp):
                    st, sp = kt == 0, kt == _ST - 1
                    nc.tensor.matmul(
                        o_pair[0:65, 0:512], vone[:, kt, 0:65], pp[:, 0:512],
                        start=st, stop=sp,
                    )
                    nc.tensor.matmul(
                        o_pair[0:65, 512:1024], vone[:, kt, 65:130],
                        pp[:, 512:1024],
                        start=st, stop=sp,
                    )

                def tail(o_pair, n):
                    # normalize tail: no PE involvement (Pool broadcast)
                    rs = nrm_pool.tile([1, 1024], F32, tag="rs")
                    nc.vector.tensor_copy(rs, o_pair[64:65, :])
                    rr = nrm_pool.tile([1, 1024], F32, tag="rr")
                    nc.vector.reciprocal_approx_fast(rr, rs)
                    rb = nrm_pool.tile([64, 1024], F32, tag="rb")
                    nc.gpsimd.partition_broadcast(rb, rr[0:1, :])
                    at = nrm_pool.tile([64, 1024], BF16, tag="at")
                    nc.vector.tensor_mul(at, o_pair[0:64, :], rb)
                    nn, half = divmod(n, 2)
                    csl = slice(half * 512, (half + 1) * 512)
                    nc.sync.dma_start(out=at_loc[p][nn][0:64, csl], in_=at[:, 0:512])
                    nc.sync.dma_start(
                        out=at_loc[p][nn][64:128, csl], in_=at[:, 512:1024]
                    )
                    if half == 1:
                        nc.gpsimd.collective_compute(
                            "AllGather",
                            mybir.AluOpType.bypass,
                            replica_groups=[[0, 1, 2, 3], [4, 5, 6, 7]],
                            ins=[at_loc[p][nn][:, :]],
                            outs=[at_full[p][nn][:, :]],
                        )

                pend = []

                def drain(limit):
                    while len(pend) > limit:
                        o_pair, n, g, opps = pend.pop(0)
                        pv(o_pair, 2 * g, opps[0])
                        pv(o_pair, 2 * g + 1, opps[1])
                        if g == _ST // 2 - 1:
                            tail(o_pair, n)

                # Q projection for pair p (p>=1; pair 0 was done in phase A)
                # via the s ring, between pairs.
                if p >= 1:
                    psl = slice(p * 128, (p + 1) * 128)
                    for nq2 in range(2):
                        psq = ps_b.tile([128, 1024], F32, tag="s", bufs=2)
                        for kc in range(_KC):
                            st, sp = kc == 0, kc == _KC - 1
                            for h in range(2):
                                nq = 2 * nq2 + h
                                nc.tensor.matmul(
                                    psq[:, h * 512 : (h + 1) * 512],
                                    wq_sb[:, p, kc, :],
                                    xt_sb[:, kc, nq * 512 : (nq + 1) * 512],
                                    start=st,
                                    stop=sp,
                                )
                        nc.vector.tensor_copy(
                            qt_sb[p][:, nq2 * 1024 : (nq2 + 1) * 1024], psq
                        )

                for n in range(_NT):
                    nsl = slice(n * 512, (n + 1) * 512)
                    o_pair = ps_b.tile([128, 1024], F32, tag="o", bufs=2)
                    for g in range(_ST // 2):
                        pps = []
                        for h in range(2):
                            kt = 2 * g + h
                            ksl = slice(kt * 128, (kt + 1) * 128)
                            s_pair = ps_b.tile([128, 1024], F32, tag="s", bufs=2)
                            nc.tensor.matmul(
                                s_pair[:, 0:512],
                                kt_sb[qsl_A, ksl],
                                qt_sb[p][qsl_A, nsl],
                                start=True,
                                stop=True,
                                tile_position=(0, 0),
                            )
                            nc.tensor.matmul(
                                s_pair[:, 512:1024],
                                kt_sb[qsl_B, ksl],
                                qt_sb[p][qsl_B, nsl],
                                start=True,
                                stop=True,
                                tile_position=(64, 0),
                            )
                            pp = pp_pool.tile([128, 1024], BF16, tag="pp")
                            nc.scalar.activation(
                                pp, s_pair, mybir.ActivationFunctionType.Exp,
                                scale=0.125,
                            )
                            pps.append(pp)
                        pend.append((o_pair, n, g, pps))
                        drain(2)
                drain(0)

        # ---- phase C: output projection (Y^T = Wo_c^T @ A^T_full) -------
        # contraction chunk kc = 4*p + cp reads at_full[p][n//2] rows
        # [128cp, 128cp+128), cols (n%2)*512 slice; pair order 0..3 puts the
        # last gather's consumers ~10us into each n-group.
        with (
            tc.tile_pool(name="ac_pool", bufs=4) as ac_pool,
            tc.tile_pool(name="y_pool", bufs=2) as y_pool,
            tc.tile_pool(name="y_ps", bufs=2, space="PSUM") as y_ps,
        ):
            for n in range(_NT):
                nsl = slice(n * 512, (n + 1) * 512)
                nn, half = divmod(n, 2)
                csl = slice(half * 512, (half + 1) * 512)
                psy = [
                    y_ps.tile([128, 512], F32, tag=f"psy{m}", name=f"psy{m}")
                    for m in range(4)
                ]
                for p in range(_NPAIR):
                    ac_t = ac_pool.tile([128, 4, 512], BF16, tag="ac")
                    nc.sync.dma_start(
                        out=ac_t,
                        in_=at_full[p][nn][:, csl].rearrange(
                            "(cp q) c -> q cp c", q=128
                        ),
                    )
                    for cp in range(4):
                        kc = 4 * p + cp
                        st, sp = kc == 0, kc == _KC - 1
                        for m in range(4):
                            nc.tensor.matmul(
                                psy[m],
                                wo_sb[:, kc, m * 128 : (m + 1) * 128],
                                ac_t[:, cp, :],
                                start=st,
                                stop=sp,
                            )
                for m in range(4):
                    y_sb = y_pool.tile([128, 512], F32, tag="y")
                    nc.vector.tensor_copy(y_sb, psy[m])
                    nc.sync.dma_start(
                        out=yt_d[m * 128 : (m + 1) * 128, nsl], in_=y_sb
                    )

    nc.compile()
    return nc


def _f32_to_bf16_bits(x: np.ndarray) -> np.ndarray:
    """Round-to-nearest-even fp32 -> bf16 bit pattern (uint16)."""
    u = np.ascontiguousarray(x, dtype=np.float32).view(np.uint32)
    r = (u + np.uint32(0x7FFF) + ((u >> np.uint32(16)) & np.uint32(1))) >> np.uint32(16)
    return r.astype(np.uint16)


def _pmajor(w_bits: np.ndarray) -> np.ndarray:
    """[2048, M] -> partition-major [128, 16*M] (kc-chunked rows)."""
    m = w_bits.shape[1]
    return np.ascontiguousarray(
        w_bits.reshape(_KC, 128, m).transpose(1, 0, 2).reshape(128, _KC * m)
    )


def _pmajor_pairs(w_bits: np.ndarray) -> np.ndarray:
    """[2048, 512] -> [128, pair*kc*128]: per-pair contiguous loads."""
    return np.ascontiguousarray(
        w_bits.reshape(_KC, 128, _NPAIR, 128)
        .transpose(1, 2, 0, 3)
        .reshape(128, _NPAIR * _KC * 128)
    )


def kernel(X, Wq, Wk, Wv, Wo):
    global _nc_cache, _last_results
    X = np.ascontiguousarray(np.asarray(X, dtype=np.float32))
    Wq = np.asarray(Wq, dtype=np.float32)
    Wk = np.asarray(Wk, dtype=np.float32)
    Wv = np.asarray(Wv, dtype=np.float32)
    Wo = np.asarray(Wo, dtype=np.float32)

    if _nc_cache is None:
        _nc_cache = _build()
    nc = _nc_cache

    xts = [_pmajor(_f32_to_bf16_bits(X[b].T)) for b in range(_BATCH)]
    # at_full row order: pair-major, then gather rank (core j), then (p, p+4)
    perm_rows = []
    for p in range(_NPAIR):
        for j in range(4):
            for h in (8 * j + p, 8 * j + p + 4):
                perm_rows.extend(range(h * _HD, (h + 1) * _HD))
    wo_p = Wo[perm_rows, :]

    in_maps = []
    for c in range(_NCORES):
        b, j = divmod(c, 4)
        qcols = []
        for o in _ORDER:
            h = 8 * j + o
            qcols.extend(range(h * _HD, (h + 1) * _HD))
        in_maps.append(
            {
                "xt": xts[b],
                "wq": _pmajor_pairs(_f32_to_bf16_bits(Wq[:, qcols])),
                "wk": _pmajor(
                    _f32_to_bf16_bits(Wk[:, 2 * j * _HD : (2 * j + 2) * _HD])
                ),
                "wv": _pmajor(
                    _f32_to_bf16_bits(Wv[:, 2 * j * _HD : (2 * j + 2) * _HD])
                ),
                "wo": _pmajor(_f32_to_bf16_bits(wo_p[:, 512 * j : 512 * (j + 1)])),
            }
        )

    trace = bool(os.environ.get("KERNEL_TRACE"))
    res = run_bass_kernel_spmd(
        nc, in_maps, core_ids=list(range(_NCORES)), trace=trace
    )
    _last_results = res

    Y = np.empty((_BATCH, _SEQ, _HIDDEN), dtype=np.float32)
    for c in range(_NCORES):
        b, j = divmod(c, 4)
        Y[b][:, 512 * j : 512 * (j + 1)] = res.results[c]["yt"].T
    return Y
